# revision 24
# baseline (speedup 1.0000x reference)
"""Trainium2 Bass kernel for span-attention pooling.

Problem shapes (hardcoded):
  x: [B=2, T=512, E=1024] f32, W: [1024, 1] f32, b: [1] f32,
  start/end: [S=2048] i32.  Output: [B, S, E] f32.

Math: out[b,s,:] = sum_{t=start[s]}^{end[s]} q[b,t] * x[b,t,:] / sum q[b,t]
with q = exp(relu(x @ W + b)).  (Equivalent to the reference's per-span
softmax over head scores, since spans are contiguous token ranges and
clamped/invalid positions carry zero weight.)

Sharding: spans are sorted by start on the host and split into 8 groups
of 256; core g handles group g for BOTH batches. A group's spans live in
a token window of ~<=100 tokens, so each core contracts a single
128-token window per batch: every pooling matmul is a one-shot
[128tok x 128span] x [128tok x 512] with no K accumulation. If an
exotic span distribution needs a bigger window, tch grows to
ceil(window/128) and the matmuls accumulate.

Per core: head scores h = x.W run on the PE (8 accumulating matmuls per
window against a host-transposed copy of the window, keeping the DVE
free and warming the PE), q = exp(h+b) (ACT, from PSUM), masked span
weights mq = max(smask*q, smask) = smask*exp(relu(h+b)) (DVE; smask is
the host-prepared 0/1 start<=t<=end mask), pooled sums po = mq^T @ x
and normalizer Z = mq^T @ 1 (PE), out = po/Z downcast to f16 (the
PSUM->SBUF crossing, split across ACT and DVE).

Internals and output run in fp16 (PE accumulates fp32); absmax-relative
error ~5e-4 vs the f32 reference.
"""

import numpy as np

import concourse.bass as bass
import concourse.tile as tile
from concourse import bacc, mybir
from concourse import bass_utils

B, T, E = 2, 512, 1024
S, A = 2048, 30
N_CORES = 8
SQ = S // N_CORES  # spans per core (256)
SCH = SQ // 128  # span chunks of 128 partitions (2)
EC = E // 128  # E chunks for the PE head-score dot (8)

F32 = mybir.dt.float32
F16 = mybir.dt.float16

N_WARM = 6  # dummy matmuls bridging PE idle until the head-score matmuls
XW = E + 16  # xt row: E transposed cols + w8 (EC) + bias (2) + pad


def _build_body(tc, tch, out_d, x_d, xt_d, sm_d):
    nc = tc.nc
    AF = mybir.ActivationFunctionType
    OP = mybir.AluOpType

    with (
        tc.tile_pool(name="main", bufs=1) as mainp,
        tc.tile_pool(name="psum", bufs=1, space="PSUM") as psp,
    ):
        # sync ring: the transposed windows, which carry w8+bias in
        # their tail columns so one DMA gates the whole head-score dot.
        # scalar ring: span masks then the pooling windows.
        xtT = [[None] * tch for _ in range(B)]
        for b in range(B):
            for i in range(tch):
                t_ = mainp.tile([128, XW], F16, name=f"xtT{b}_{i}", tag=f"xtT{b}_{i}")
                r = (b * tch + i) * 128
                nc.sync.dma_start(t_[:], xt_d[r : r + 128, :])
                xtT[b][i] = t_
        w8 = xtT[0][0][:, E : E + EC]
        bb = xtT[0][0][:, E + EC : E + EC + 2].bitcast(F32)
        smasks = []
        for i in range(tch):
            sm = mainp.tile([128, SQ], F16, name=f"sm{i}", tag=f"sm{i}")
            nc.scalar.dma_start(sm[:], sm_d[128 * i : 128 * (i + 1), :])
            smasks.append(sm)
        # ones on GpSimd (its queue is free right after the preamble):
        # Z-matmul rhs (first 64 cols) + PE warm-up operand.
        ones16 = mainp.tile([128, 512], F16)
        nc.gpsimd.memset(ones16[:], 1.0)

        # Pooling windows: not needed until the pools (~2us after the
        # transposed windows), so a GpSimd memset of each tile makes the
        # DMA wait (write-after-write) and keeps early HBM bandwidth for
        # the critical transposed-window loads.
        xts = [[None] * tch for _ in range(B)]
        for b in range(B):
            for i in range(tch):
                xt = mainp.tile([128, E], F16, name=f"xt{b}_{i}", tag=f"xt{b}_{i}")
                r = (b * tch + i) * 128
                nc.gpsimd.memset(xt[:], 0.0)
                nc.scalar.dma_start(xt[:], x_d[r : r + 128, :])
                xts[b][i] = xt

        # Dummy matmuls bridge the PE from program start until the
        # head-score matmuls so the HAM clock gate releases early.
        # (The warm bank is recycled as t0's poA later.)
        warm = psp.tile([128, 512], F32, name="warm", tag="pA0")
        for _ in range(N_WARM):
            nc.tensor.matmul(
                warm[:], ones16[:, 0:128], ones16[:], start=True, stop=True
            )

        # Head scores on the PE: h[t] = sum_c xT_chunk[c].T @ w8[:, c].
        # h lands in PSUM (a separate bank per batch so exp_b0's read
        # doesn't serialize h_b1's write under tile-granular tracking);
        # q = exp(h + bias) on ACT reads PSUM directly.
        # full-bank h tiles so tile 2's po accumulators can recycle the
        # hb banks (free once exp has read them)
        hps = [
            psp.tile([128, 512], F32, name=f"hp{b}", tag=f"hb{b}") for b in range(B)
        ]
        qs = [mainp.tile([128, tch], F32, name=f"q{b}") for b in range(B)]
        mqs = [[None] * tch for _ in range(B)]
        for b in range(B):
            for i in range(tch):
                for ec in range(EC):
                    nc.tensor.matmul(
                        hps[b][:, i : i + 1],
                        xtT[b][i][:, 128 * ec : 128 * (ec + 1)],
                        w8[:, ec : ec + 1],
                        start=(ec == 0),
                        stop=(ec == EC - 1),
                    )
                with tc.high_priority():
                    nc.scalar.activation(
                        qs[b][:, i : i + 1], hps[b][:, i : i + 1], AF.Exp, bias=bb
                    )
                    # mq = max(smask*q, smask) = smask * exp(relu(h+b))
                    mq = mainp.tile([128, SQ], F16, name=f"mq{b}_{i}", tag=f"mq{b}_{i}")
                    nc.vector.scalar_tensor_tensor(
                        mq[:],
                        smasks[i][:],
                        qs[b][:, i : i + 1],
                        smasks[i][:],
                        op0=OP.mult,
                        op1=OP.max,
                    )
                mqs[b][i] = mq

        # Pooling matmuls, one output tile per (batch, span-chunk):
        #   po[s, e] = sum_t mq[t, s] * x[t, e];  Z[s] = sum_t mq[t, s]
        # PSUM banks: hb0 hb1 + z01 z23 + 2 (poA,poB) pairs = 8; warm-up
        # borrowed pA0 (done long before t0's poA).  Z's pair up in one
        # bank per two tiles with a single strided reciprocal for both.
        zts = [
            psp.tile([128, 128], F32, name=f"z{p}", tag=f"zb{p}") for p in range(2)
        ]
        po_tags = [("pA0", "pB0"), ("pA1", "pB1"), ("hb0", "hb1"), ("pA0", "pB0")]

        for zp in range(2):  # pair p covers tiles (2p, 2p+1), same batch
            b = zp
            # both Z's first: their reciprocals come before any po
            # matmul of the pair, and the shared z bank sees all writes
            # before its readers (no write-after-read stall on the PE).
            for zc in range(2):
                zsl = zts[zp][:, 64 * zc : 64 * zc + 64]
                for i in range(tch):
                    lhsT = mqs[b][i][:, 128 * zc : 128 * (zc + 1)]
                    nc.tensor.matmul(
                        zsl, lhsT, ones16[:, 0:64],
                        start=(i == 0), stop=(i == tch - 1),
                    )
            rzp = []
            with tc.high_priority():
                for zc in range(2):
                    u = 2 * zp + zc
                    rz = mainp.tile([128, 1], F32, name=f"rz{u}", tag=f"rz{u}")
                    nc.vector.reciprocal(rz[:], zts[zp][:, 64 * zc : 64 * zc + 1])
                    rzp.append(rz)
            for zc in range(2):
                u = 2 * zp + zc
                tagA, tagB = po_tags[u]
                poA = psp.tile([128, 512], F32, name=f"poA{u}", tag=tagA)
                poB = psp.tile([128, 512], F32, name=f"poB{u}", tag=tagB)
                for i in range(tch):
                    st_, sp_ = (i == 0), (i == tch - 1)
                    lhsT = mqs[b][i][:, 128 * zc : 128 * (zc + 1)]
                    nc.tensor.matmul(
                        poA[:], lhsT, xts[b][i][:, 0:512], start=st_, stop=sp_
                    )
                    nc.tensor.matmul(
                        poB[:], lhsT, xts[b][i][:, 512:1024], start=st_, stop=sp_
                    )
                with tc.high_priority():
                    ob = mainp.tile([128, E], F16, name=f"ob{u}", tag=f"ob{u}")
                    nc.scalar.mul(ob[:, 0:512], poA[:], rzp[zc][:])
                    nc.vector.tensor_scalar_mul(ob[:, 512:1024], poB[:], rzp[zc][:])
                dma_eng = nc.gpsimd if u < 3 else nc.sync
                r = 128 * u
                dma_eng.dma_start(out_d[r : r + 128, :], ob[:])


def _build(tch):
    nc = bacc.Bacc(
        "TRN2",
        target_bir_lowering=False,
        debug=False,
        num_devices=N_CORES,
    )
    x_d = nc.dram_tensor("x", [B * tch * 128, E], F16, kind="ExternalInput").ap()
    xt_d = nc.dram_tensor("xt", [B * tch * 128, XW], F16, kind="ExternalInput").ap()
    sm_d = nc.dram_tensor("sm", [tch * 128, SQ], F16, kind="ExternalInput").ap()
    out_d = nc.dram_tensor("out", [B * SQ, E], F16, kind="ExternalOutput").ap()
    with tile.TileContext(nc) as tc:
        _build_body(tc, tch, out_d, x_d, xt_d, sm_d)
    nc.compile()
    return nc


_NC_CACHE = {}


def _get_nc(tch):
    if tch not in _NC_CACHE:
        _NC_CACHE[tch] = _build(tch)
    return _NC_CACHE[tch]


def _make_in_maps(tch, x, W, b, start, end, groups, los):
    """groups[g] = span indices for core g; los[g] = first token of g's
    x window. Each group has exactly SQ spans whose tokens fit in
    [los[g], los[g] + 128*tch)."""
    x = np.asarray(x, dtype=np.float32)
    start = np.asarray(start, dtype=np.int32)
    end = np.asarray(end, dtype=np.int32)
    # w8[p, c] = W[c*128 + p]; then bias as f32 bits (packed into xt tail)
    w8b = np.zeros((128, 16), np.float16)
    w8b[:, 0:EC] = np.asarray(W, np.float32).reshape(EC, 128).T.astype(np.float16)
    w8b[:, EC : EC + 2] = np.asarray(b, np.float32).reshape(1).view(np.float16)[
        None, :
    ]
    nrow = 128 * tch
    toks = np.arange(nrow, dtype=np.int64)
    in_maps = []
    for g in range(N_CORES):
        idx = groups[g]
        lo = los[g]
        tok_ids = lo + toks  # [nrow]
        sm = (
            (start[idx][None, :] <= tok_ids[:, None])
            & (tok_ids[:, None] <= end[idx][None, :])
        ).astype(np.float16)
        xw = np.zeros((B * nrow, E), np.float16)
        hi = min(lo + nrow, T)
        for bb_idx in range(B):
            xw[bb_idx * nrow : bb_idx * nrow + hi - lo] = x[bb_idx, lo:hi].astype(
                np.float16
            )
        # xT chunks: xtw[(b*tch+i)*128 + p, c*128 + t] = xw[(b*tch+i)*128 + t, c*128 + p]
        xtw = np.empty((B * tch * 128, XW), np.float16)
        xtw[:, 0:E] = (
            xw.reshape(B * tch, 128, EC, 128)
            .transpose(0, 3, 2, 1)
            .reshape(B * tch * 128, E)
        )
        xtw[:, E:XW] = np.tile(w8b, (B * tch, 1))
        in_maps.append(
            {
                "x": np.ascontiguousarray(xw),
                "xt": np.ascontiguousarray(xtw),
                "sm": np.ascontiguousarray(sm),
            }
        )
    return in_maps


def run(x, W, b, start, end, trace=False, trace_cores=None):
    """Run on 8 cores; returns (out[B,S,E] f32, BassKernelResults)."""
    start_np = np.asarray(start, dtype=np.int32)
    end_np = np.asarray(end, dtype=np.int32)

    # Windowed sharding: sort spans by start, split into 8 groups of 256.
    order = np.argsort(start_np, kind="stable")
    groups = [order[g * SQ : (g + 1) * SQ] for g in range(N_CORES)]
    los, wmax = [], 1
    for idx in groups:
        lo = int(start_np[idx].min())
        hi = max(int(end_np[idx].max()), lo)
        los.append(min(lo, T - 1))
        wmax = max(wmax, hi - lo + 1)
    tch = (wmax + 127) // 128
    assert tch <= (T + 127) // 128

    nc = _get_nc(tch)
    in_maps = _make_in_maps(tch, x, W, b, start, end, groups, los)
    res = bass_utils.run_bass_kernel_spmd(
        nc,
        in_maps,
        core_ids=list(range(N_CORES)),
        trace=trace,
        trace_cores=trace_cores,
    )
    out = np.empty((B, S, E), np.float32)
    for g in range(N_CORES):
        out[:, groups[g], :] = (
            res.results[g]["out"].astype(np.float32).reshape(B, SQ, E)
        )
    return out, res


def kernel(x, W, b, start, end):
    out, _ = run(x, W, b, start, end, trace=False)
    return out


# revision 25
# speedup vs baseline: 1.0004x; 1.0004x over previous
"""Trainium2 Bass kernel for span-attention pooling.

Problem shapes (hardcoded):
  x: [B=2, T=512, E=1024] f32, W: [1024, 1] f32, b: [1] f32,
  start/end: [S=2048] i32.  Output: [B, S, E] f32.

Math: out[b,s,:] = sum_{t=start[s]}^{end[s]} q[b,t] * x[b,t,:] / sum q[b,t]
with q = exp(relu(x @ W + b)).  (Equivalent to the reference's per-span
softmax over head scores, since spans are contiguous token ranges and
clamped/invalid positions carry zero weight.)

Sharding: spans are sorted by start on the host and split into 8 groups
of 256; core g handles group g for BOTH batches. A group's spans live in
a token window of ~<=100 tokens, so each core contracts a single
128-token window per batch: every pooling matmul is a one-shot
[128tok x 128span] x [128tok x 512] with no K accumulation. If an
exotic span distribution needs a bigger window, tch grows to
ceil(window/128) and the matmuls accumulate.

Per core: head scores h = x.W run on the PE (8 accumulating matmuls per
window against a host-transposed copy of the window, keeping the DVE
free and warming the PE), q = exp(h+b) (ACT, from PSUM), masked span
weights mq = max(smask*q, smask) = smask*exp(relu(h+b)) (DVE; smask is
the host-prepared 0/1 start<=t<=end mask), pooled sums po = mq^T @ x
and normalizer Z = mq^T @ 1 (PE), out = po/Z downcast to f16 (the
PSUM->SBUF crossing, split across ACT and DVE).

Internals and output run in fp16 (PE accumulates fp32); absmax-relative
error ~5e-4 vs the f32 reference.
"""

import numpy as np

import concourse.bass as bass
import concourse.tile as tile
from concourse import bacc, mybir
from concourse import bass_utils

B, T, E = 2, 512, 1024
S, A = 2048, 30
N_CORES = 8
SQ = S // N_CORES  # spans per core (256)
SCH = SQ // 128  # span chunks of 128 partitions (2)
EC = E // 128  # E chunks for the PE head-score dot (8)

F32 = mybir.dt.float32
F16 = mybir.dt.float16

N_WARM = 6  # dummy matmuls bridging PE idle until the head-score matmuls
XW = E + 16  # xt row: E transposed cols + w8 (EC) + bias (2) + pad


def _build_body(tc, tch, out_d, x_d, xt_d, sm_d):
    nc = tc.nc
    AF = mybir.ActivationFunctionType
    OP = mybir.AluOpType

    with (
        tc.tile_pool(name="main", bufs=1) as mainp,
        tc.tile_pool(name="psum", bufs=1, space="PSUM") as psp,
    ):
        # sync ring: the transposed windows, which carry w8+bias in
        # their tail columns so one DMA gates the whole head-score dot.
        # scalar ring: span masks then the pooling windows.
        xtT = [[None] * tch for _ in range(B)]
        for b in range(B):
            for i in range(tch):
                t_ = mainp.tile([128, XW], F16, name=f"xtT{b}_{i}", tag=f"xtT{b}_{i}")
                r = (b * tch + i) * 128
                nc.sync.dma_start(t_[:], xt_d[r : r + 128, :])
                xtT[b][i] = t_
        w8 = xtT[0][0][:, E : E + EC]
        bb = xtT[0][0][:, E + EC : E + EC + 2].bitcast(F32)
        smasks = []
        for i in range(tch):
            sm = mainp.tile([128, SQ], F16, name=f"sm{i}", tag=f"sm{i}")
            nc.scalar.dma_start(sm[:], sm_d[128 * i : 128 * (i + 1), :])
            smasks.append(sm)
        # ones on GpSimd (its queue is free right after the preamble):
        # Z-matmul rhs (first 64 cols) + PE warm-up operand.
        ones16 = mainp.tile([128, 512], F16)
        nc.gpsimd.memset(ones16[:], 1.0)

        # Pooling windows: not needed until the pools (~2us after the
        # transposed windows), so a GpSimd memset of each tile makes the
        # DMA wait (write-after-write) and keeps early HBM bandwidth for
        # the critical transposed-window loads.
        xts = [[None] * tch for _ in range(B)]
        for b in range(B):
            for i in range(tch):
                xt = mainp.tile([128, E], F16, name=f"xt{b}_{i}", tag=f"xt{b}_{i}")
                r = (b * tch + i) * 128
                # small marker: delays the DMA ~0.3us per tile without
                # starving the pools
                nc.gpsimd.memset(xt[:, 0:256], 0.0)
                nc.scalar.dma_start(xt[:], x_d[r : r + 128, :])
                xts[b][i] = xt

        # Dummy matmuls bridge the PE from program start until the
        # head-score matmuls so the HAM clock gate releases early.
        # (The warm bank is recycled as t0's poA later.)
        warm = psp.tile([128, 512], F32, name="warm", tag="pA0")
        for _ in range(N_WARM):
            nc.tensor.matmul(
                warm[:], ones16[:, 0:128], ones16[:], start=True, stop=True
            )

        # Head scores on the PE: h[t] = sum_c xT_chunk[c].T @ w8[:, c].
        # h lands in PSUM (a separate bank per batch so exp_b0's read
        # doesn't serialize h_b1's write under tile-granular tracking);
        # q = exp(h + bias) on ACT reads PSUM directly.
        # full-bank h tiles so tile 2's po accumulators can recycle the
        # hb banks (free once exp has read them)
        hps = [
            psp.tile([128, 512], F32, name=f"hp{b}", tag=f"hb{b}") for b in range(B)
        ]
        qs = [mainp.tile([128, tch], F32, name=f"q{b}") for b in range(B)]
        mqs = [[None] * tch for _ in range(B)]
        for b in range(B):
            for i in range(tch):
                for ec in range(EC):
                    nc.tensor.matmul(
                        hps[b][:, i : i + 1],
                        xtT[b][i][:, 128 * ec : 128 * (ec + 1)],
                        w8[:, ec : ec + 1],
                        start=(ec == 0),
                        stop=(ec == EC - 1),
                    )
                with tc.high_priority():
                    nc.scalar.activation(
                        qs[b][:, i : i + 1], hps[b][:, i : i + 1], AF.Exp, bias=bb
                    )
                    # mq = max(smask*q, smask) = smask * exp(relu(h+b))
                    mq = mainp.tile([128, SQ], F16, name=f"mq{b}_{i}", tag=f"mq{b}_{i}")
                    nc.vector.scalar_tensor_tensor(
                        mq[:],
                        smasks[i][:],
                        qs[b][:, i : i + 1],
                        smasks[i][:],
                        op0=OP.mult,
                        op1=OP.max,
                    )
                mqs[b][i] = mq

        # Pooling matmuls, one output tile per (batch, span-chunk):
        #   po[s, e] = sum_t mq[t, s] * x[t, e];  Z[s] = sum_t mq[t, s]
        # PSUM banks: hb0 hb1 + z01 z23 + 2 (poA,poB) pairs = 8; warm-up
        # borrowed pA0 (done long before t0's poA).  Z's pair up in one
        # bank per two tiles with a single strided reciprocal for both.
        zts = [
            psp.tile([128, 128], F32, name=f"z{p}", tag=f"zb{p}") for p in range(2)
        ]
        po_tags = [("pA0", "pB0"), ("pA1", "pB1"), ("hb0", "hb1"), ("pA0", "pB0")]

        for zp in range(2):  # pair p covers tiles (2p, 2p+1), same batch
            b = zp
            # both Z's first: their reciprocals come before any po
            # matmul of the pair, and the shared z bank sees all writes
            # before its readers (no write-after-read stall on the PE).
            for zc in range(2):
                zsl = zts[zp][:, 64 * zc : 64 * zc + 64]
                for i in range(tch):
                    lhsT = mqs[b][i][:, 128 * zc : 128 * (zc + 1)]
                    nc.tensor.matmul(
                        zsl, lhsT, ones16[:, 0:64],
                        start=(i == 0), stop=(i == tch - 1),
                    )
            rzp = []
            with tc.high_priority():
                for zc in range(2):
                    u = 2 * zp + zc
                    rz = mainp.tile([128, 1], F32, name=f"rz{u}", tag=f"rz{u}")
                    nc.vector.reciprocal(rz[:], zts[zp][:, 64 * zc : 64 * zc + 1])
                    rzp.append(rz)
            for zc in range(2):
                u = 2 * zp + zc
                tagA, tagB = po_tags[u]
                poA = psp.tile([128, 512], F32, name=f"poA{u}", tag=tagA)
                poB = psp.tile([128, 512], F32, name=f"poB{u}", tag=tagB)
                for i in range(tch):
                    st_, sp_ = (i == 0), (i == tch - 1)
                    lhsT = mqs[b][i][:, 128 * zc : 128 * (zc + 1)]
                    nc.tensor.matmul(
                        poA[:], lhsT, xts[b][i][:, 0:512], start=st_, stop=sp_
                    )
                    nc.tensor.matmul(
                        poB[:], lhsT, xts[b][i][:, 512:1024], start=st_, stop=sp_
                    )
                with tc.high_priority():
                    ob = mainp.tile([128, E], F16, name=f"ob{u}", tag=f"ob{u}")
                    nc.scalar.mul(ob[:, 0:512], poA[:], rzp[zc][:])
                    nc.vector.tensor_scalar_mul(ob[:, 512:1024], poB[:], rzp[zc][:])
                dma_eng = nc.gpsimd if u < 3 else nc.sync
                r = 128 * u
                dma_eng.dma_start(out_d[r : r + 128, :], ob[:])


def _build(tch):
    nc = bacc.Bacc(
        "TRN2",
        target_bir_lowering=False,
        debug=False,
        num_devices=N_CORES,
    )
    x_d = nc.dram_tensor("x", [B * tch * 128, E], F16, kind="ExternalInput").ap()
    xt_d = nc.dram_tensor("xt", [B * tch * 128, XW], F16, kind="ExternalInput").ap()
    sm_d = nc.dram_tensor("sm", [tch * 128, SQ], F16, kind="ExternalInput").ap()
    out_d = nc.dram_tensor("out", [B * SQ, E], F16, kind="ExternalOutput").ap()
    with tile.TileContext(nc) as tc:
        _build_body(tc, tch, out_d, x_d, xt_d, sm_d)
    nc.compile()
    return nc


_NC_CACHE = {}


def _get_nc(tch):
    if tch not in _NC_CACHE:
        _NC_CACHE[tch] = _build(tch)
    return _NC_CACHE[tch]


def _make_in_maps(tch, x, W, b, start, end, groups, los):
    """groups[g] = span indices for core g; los[g] = first token of g's
    x window. Each group has exactly SQ spans whose tokens fit in
    [los[g], los[g] + 128*tch)."""
    x = np.asarray(x, dtype=np.float32)
    start = np.asarray(start, dtype=np.int32)
    end = np.asarray(end, dtype=np.int32)
    # w8[p, c] = W[c*128 + p]; then bias as f32 bits (packed into xt tail)
    w8b = np.zeros((128, 16), np.float16)
    w8b[:, 0:EC] = np.asarray(W, np.float32).reshape(EC, 128).T.astype(np.float16)
    w8b[:, EC : EC + 2] = np.asarray(b, np.float32).reshape(1).view(np.float16)[
        None, :
    ]
    nrow = 128 * tch
    toks = np.arange(nrow, dtype=np.int64)
    in_maps = []
    for g in range(N_CORES):
        idx = groups[g]
        lo = los[g]
        tok_ids = lo + toks  # [nrow]
        sm = (
            (start[idx][None, :] <= tok_ids[:, None])
            & (tok_ids[:, None] <= end[idx][None, :])
        ).astype(np.float16)
        xw = np.zeros((B * nrow, E), np.float16)
        hi = min(lo + nrow, T)
        for bb_idx in range(B):
            xw[bb_idx * nrow : bb_idx * nrow + hi - lo] = x[bb_idx, lo:hi].astype(
                np.float16
            )
        # xT chunks: xtw[(b*tch+i)*128 + p, c*128 + t] = xw[(b*tch+i)*128 + t, c*128 + p]
        xtw = np.empty((B * tch * 128, XW), np.float16)
        xtw[:, 0:E] = (
            xw.reshape(B * tch, 128, EC, 128)
            .transpose(0, 3, 2, 1)
            .reshape(B * tch * 128, E)
        )
        xtw[:, E:XW] = np.tile(w8b, (B * tch, 1))
        in_maps.append(
            {
                "x": np.ascontiguousarray(xw),
                "xt": np.ascontiguousarray(xtw),
                "sm": np.ascontiguousarray(sm),
            }
        )
    return in_maps


def run(x, W, b, start, end, trace=False, trace_cores=None):
    """Run on 8 cores; returns (out[B,S,E] f32, BassKernelResults)."""
    start_np = np.asarray(start, dtype=np.int32)
    end_np = np.asarray(end, dtype=np.int32)

    # Windowed sharding: sort spans by start, split into 8 groups of 256.
    order = np.argsort(start_np, kind="stable")
    groups = [order[g * SQ : (g + 1) * SQ] for g in range(N_CORES)]
    los, wmax = [], 1
    for idx in groups:
        lo = int(start_np[idx].min())
        hi = max(int(end_np[idx].max()), lo)
        los.append(min(lo, T - 1))
        wmax = max(wmax, hi - lo + 1)
    tch = (wmax + 127) // 128
    assert tch <= (T + 127) // 128

    nc = _get_nc(tch)
    in_maps = _make_in_maps(tch, x, W, b, start, end, groups, los)
    res = bass_utils.run_bass_kernel_spmd(
        nc,
        in_maps,
        core_ids=list(range(N_CORES)),
        trace=trace,
        trace_cores=trace_cores,
    )
    out = np.empty((B, S, E), np.float32)
    for g in range(N_CORES):
        out[:, groups[g], :] = (
            res.results[g]["out"].astype(np.float32).reshape(B, SQ, E)
        )
    return out, res


def kernel(x, W, b, start, end):
    out, _ = run(x, W, b, start, end, trace=False)
    return out


# revision 27
# speedup vs baseline: 1.0155x; 1.0152x over previous
"""Trainium2 Bass kernel for span-attention pooling.

Problem shapes (hardcoded):
  x: [B=2, T=512, E=1024] f32, W: [1024, 1] f32, b: [1] f32,
  start/end: [S=2048] i32.  Output: [B, S, E] f32.

Math: out[b,s,:] = sum_{t=start[s]}^{end[s]} q[b,t] * x[b,t,:] / sum q[b,t]
with q = exp(relu(x @ W + b)).  (Equivalent to the reference's per-span
softmax over head scores, since spans are contiguous token ranges and
clamped/invalid positions carry zero weight.)

Sharding: spans are sorted by start on the host and split into 8 groups
of 256; core g handles group g for BOTH batches. A group's spans live in
a token window of ~<=100 tokens, so each core contracts a single
128-token window per batch: every pooling matmul is a one-shot
[128tok x 128span] x [128tok x 512] with no K accumulation. If an
exotic span distribution needs a bigger window, tch grows to
ceil(window/128) and the matmuls accumulate.

Per core: head scores h = x.W run on the PE (8 accumulating matmuls per
window against a host-transposed copy of the window, keeping the DVE
free and warming the PE), q = exp(h+b) (ACT, from PSUM), masked span
weights mq = max(smask*q, smask) = smask*exp(relu(h+b)) (DVE; smask is
the host-prepared 0/1 start<=t<=end mask), pooled sums po = mq^T @ x
and normalizer Z = mq^T @ 1 (PE), out = po/Z downcast to f16 (the
PSUM->SBUF crossing, split across ACT and DVE).

Internals and output run in fp16 (PE accumulates fp32); absmax-relative
error ~5e-4 vs the f32 reference.
"""

import numpy as np

import concourse.bass as bass
import concourse.tile as tile
from concourse import bacc, mybir
from concourse import bass_utils

B, T, E = 2, 512, 1024
S, A = 2048, 30
N_CORES = 8
SQ = S // N_CORES  # spans per core (256)
SCH = SQ // 128  # span chunks of 128 partitions (2)
EC = E // 128  # E chunks for the PE head-score dot (8)

F32 = mybir.dt.float32
F16 = mybir.dt.float16

N_WARM = 6  # dummy matmuls bridging PE idle until the head-score matmuls
XW = E + 16  # xt row: E transposed cols + w8 (EC) + bias (2) + pad


def _build_body(tc, tch, out_d, x_d, xt_d, sm_d):
    nc = tc.nc
    AF = mybir.ActivationFunctionType
    OP = mybir.AluOpType

    with (
        tc.tile_pool(name="main", bufs=1) as mainp,
        tc.tile_pool(name="psum", bufs=1, space="PSUM") as psp,
    ):
        # sync ring: the transposed windows, which carry w8+bias in
        # their tail columns so one DMA gates the whole head-score dot.
        # scalar ring: span masks then the pooling windows.
        xtT = [[None] * tch for _ in range(B)]
        for b in range(B):
            for i in range(tch):
                t_ = mainp.tile([128, XW], F16, name=f"xtT{b}_{i}", tag=f"xtT{b}_{i}")
                r = (b * tch + i) * 128
                nc.sync.dma_start(t_[:], xt_d[r : r + 128, :])
                xtT[b][i] = t_
        w8 = xtT[0][0][:, E : E + EC]
        bb = xtT[0][0][:, E + EC : E + EC + 2].bitcast(F32)
        # ones on GpSimd (its queue is free right after the preamble):
        # Z-matmul rhs (first 64 cols) + PE warm-up operand.  The span
        # masks ride the GpSimd SWDGE queue - a third DMA queue, so they
        # sit in front of neither the transposed nor the pooling windows.
        ones16 = mainp.tile([128, 512], F16)
        nc.gpsimd.memset(ones16[:], 1.0)
        smasks = []
        for i in range(tch):
            sm = mainp.tile([128, SQ], F16, name=f"sm{i}", tag=f"sm{i}")
            nc.gpsimd.dma_start(sm[:], sm_d[128 * i : 128 * (i + 1), :])
            smasks.append(sm)

        xts = [[None] * tch for _ in range(B)]
        for b in range(B):
            for i in range(tch):
                xt = mainp.tile([128, E], F16, name=f"xt{b}_{i}", tag=f"xt{b}_{i}")
                r = (b * tch + i) * 128
                nc.scalar.dma_start(xt[:], x_d[r : r + 128, :])
                xts[b][i] = xt

        # Dummy matmuls bridge the PE from program start until the
        # head-score matmuls so the HAM clock gate releases early.
        # (The warm bank is recycled as t0's poA later.)
        warm = psp.tile([128, 512], F32, name="warm", tag="pA0")
        for _ in range(N_WARM):
            nc.tensor.matmul(
                warm[:], ones16[:, 0:128], ones16[:], start=True, stop=True
            )

        # Head scores on the PE: h[t] = sum_c xT_chunk[c].T @ w8[:, c].
        # h lands in PSUM (a separate bank per batch so exp_b0's read
        # doesn't serialize h_b1's write under tile-granular tracking);
        # q = exp(h + bias) on ACT reads PSUM directly.
        # full-bank h tiles so tile 2's po accumulators can recycle the
        # hb banks (free once exp has read them)
        hps = [
            psp.tile([128, 512], F32, name=f"hp{b}", tag=f"hb{b}") for b in range(B)
        ]
        qs = [mainp.tile([128, tch], F32, name=f"q{b}") for b in range(B)]
        mqs = [[None] * tch for _ in range(B)]
        for b in range(B):
            for i in range(tch):
                for ec in range(EC):
                    nc.tensor.matmul(
                        hps[b][:, i : i + 1],
                        xtT[b][i][:, 128 * ec : 128 * (ec + 1)],
                        w8[:, ec : ec + 1],
                        start=(ec == 0),
                        stop=(ec == EC - 1),
                    )
                with tc.high_priority():
                    nc.scalar.activation(
                        qs[b][:, i : i + 1], hps[b][:, i : i + 1], AF.Exp, bias=bb
                    )
                    # mq = max(smask*q, smask) = smask * exp(relu(h+b))
                    mq = mainp.tile([128, SQ], F16, name=f"mq{b}_{i}", tag=f"mq{b}_{i}")
                    nc.vector.scalar_tensor_tensor(
                        mq[:],
                        smasks[i][:],
                        qs[b][:, i : i + 1],
                        smasks[i][:],
                        op0=OP.mult,
                        op1=OP.max,
                    )
                mqs[b][i] = mq

        # Pooling matmuls, one output tile per (batch, span-chunk):
        #   po[s, e] = sum_t mq[t, s] * x[t, e];  Z[s] = sum_t mq[t, s]
        # PSUM banks: hb0 hb1 + z01 z23 + 2 (poA,poB) pairs = 8; warm-up
        # borrowed pA0 (done long before t0's poA).  Z's pair up in one
        # bank per two tiles with a single strided reciprocal for both.
        zts = [
            psp.tile([128, 128], F32, name=f"z{p}", tag=f"zb{p}") for p in range(2)
        ]
        po_tags = [("pA0", "pB0"), ("pA1", "pB1"), ("hb0", "hb1"), ("pA0", "pB0")]

        for zp in range(2):  # pair p covers tiles (2p, 2p+1), same batch
            b = zp
            # both Z's first: their reciprocals come before any po
            # matmul of the pair, and the shared z bank sees all writes
            # before its readers (no write-after-read stall on the PE).
            for zc in range(2):
                zsl = zts[zp][:, 64 * zc : 64 * zc + 64]
                for i in range(tch):
                    lhsT = mqs[b][i][:, 128 * zc : 128 * (zc + 1)]
                    nc.tensor.matmul(
                        zsl, lhsT, ones16[:, 0:64],
                        start=(i == 0), stop=(i == tch - 1),
                    )
            rzp = []
            with tc.high_priority():
                for zc in range(2):
                    u = 2 * zp + zc
                    rz = mainp.tile([128, 1], F32, name=f"rz{u}", tag=f"rz{u}")
                    nc.vector.reciprocal(rz[:], zts[zp][:, 64 * zc : 64 * zc + 1])
                    rzp.append(rz)
            for zc in range(2):
                u = 2 * zp + zc
                tagA, tagB = po_tags[u]
                poA = psp.tile([128, 512], F32, name=f"poA{u}", tag=tagA)
                poB = psp.tile([128, 512], F32, name=f"poB{u}", tag=tagB)
                for i in range(tch):
                    st_, sp_ = (i == 0), (i == tch - 1)
                    lhsT = mqs[b][i][:, 128 * zc : 128 * (zc + 1)]
                    nc.tensor.matmul(
                        poA[:], lhsT, xts[b][i][:, 0:512], start=st_, stop=sp_
                    )
                    nc.tensor.matmul(
                        poB[:], lhsT, xts[b][i][:, 512:1024], start=st_, stop=sp_
                    )
                with tc.high_priority():
                    ob = mainp.tile([128, E], F16, name=f"ob{u}", tag=f"ob{u}")
                    nc.scalar.mul(ob[:, 0:512], poA[:], rzp[zc][:])
                    # ACT takes one B-half (t0's) to balance the DVE,
                    # which also carries the masks and reciprocals
                    if u == 0:
                        nc.scalar.mul(ob[:, 512:1024], poB[:], rzp[zc][:])
                    else:
                        nc.vector.tensor_scalar_mul(
                            ob[:, 512:1024], poB[:], rzp[zc][:]
                        )
                dma_eng = nc.gpsimd if u < 3 else nc.sync
                r = 128 * u
                dma_eng.dma_start(out_d[r : r + 128, :], ob[:])


def _build(tch):
    nc = bacc.Bacc(
        "TRN2",
        target_bir_lowering=False,
        debug=False,
        num_devices=N_CORES,
    )
    x_d = nc.dram_tensor("x", [B * tch * 128, E], F16, kind="ExternalInput").ap()
    xt_d = nc.dram_tensor("xt", [B * tch * 128, XW], F16, kind="ExternalInput").ap()
    sm_d = nc.dram_tensor("sm", [tch * 128, SQ], F16, kind="ExternalInput").ap()
    out_d = nc.dram_tensor("out", [B * SQ, E], F16, kind="ExternalOutput").ap()
    with tile.TileContext(nc) as tc:
        _build_body(tc, tch, out_d, x_d, xt_d, sm_d)
    nc.compile()
    return nc


_NC_CACHE = {}


def _get_nc(tch):
    if tch not in _NC_CACHE:
        _NC_CACHE[tch] = _build(tch)
    return _NC_CACHE[tch]


def _make_in_maps(tch, x, W, b, start, end, groups, los):
    """groups[g] = span indices for core g; los[g] = first token of g's
    x window. Each group has exactly SQ spans whose tokens fit in
    [los[g], los[g] + 128*tch)."""
    x = np.asarray(x, dtype=np.float32)
    start = np.asarray(start, dtype=np.int32)
    end = np.asarray(end, dtype=np.int32)
    # w8[p, c] = W[c*128 + p]; then bias as f32 bits (packed into xt tail)
    w8b = np.zeros((128, 16), np.float16)
    w8b[:, 0:EC] = np.asarray(W, np.float32).reshape(EC, 128).T.astype(np.float16)
    w8b[:, EC : EC + 2] = np.asarray(b, np.float32).reshape(1).view(np.float16)[
        None, :
    ]
    nrow = 128 * tch
    toks = np.arange(nrow, dtype=np.int64)
    in_maps = []
    for g in range(N_CORES):
        idx = groups[g]
        lo = los[g]
        tok_ids = lo + toks  # [nrow]
        sm = (
            (start[idx][None, :] <= tok_ids[:, None])
            & (tok_ids[:, None] <= end[idx][None, :])
        ).astype(np.float16)
        xw = np.zeros((B * nrow, E), np.float16)
        hi = min(lo + nrow, T)
        for bb_idx in range(B):
            xw[bb_idx * nrow : bb_idx * nrow + hi - lo] = x[bb_idx, lo:hi].astype(
                np.float16
            )
        # xT chunks: xtw[(b*tch+i)*128 + p, c*128 + t] = xw[(b*tch+i)*128 + t, c*128 + p]
        xtw = np.empty((B * tch * 128, XW), np.float16)
        xtw[:, 0:E] = (
            xw.reshape(B * tch, 128, EC, 128)
            .transpose(0, 3, 2, 1)
            .reshape(B * tch * 128, E)
        )
        xtw[:, E:XW] = np.tile(w8b, (B * tch, 1))
        in_maps.append(
            {
                "x": np.ascontiguousarray(xw),
                "xt": np.ascontiguousarray(xtw),
                "sm": np.ascontiguousarray(sm),
            }
        )
    return in_maps


def run(x, W, b, start, end, trace=False, trace_cores=None):
    """Run on 8 cores; returns (out[B,S,E] f32, BassKernelResults)."""
    start_np = np.asarray(start, dtype=np.int32)
    end_np = np.asarray(end, dtype=np.int32)

    # Windowed sharding: sort spans by start, split into 8 groups of 256.
    order = np.argsort(start_np, kind="stable")
    groups = [order[g * SQ : (g + 1) * SQ] for g in range(N_CORES)]
    los, wmax = [], 1
    for idx in groups:
        lo = int(start_np[idx].min())
        hi = max(int(end_np[idx].max()), lo)
        los.append(min(lo, T - 1))
        wmax = max(wmax, hi - lo + 1)
    tch = (wmax + 127) // 128
    assert tch <= (T + 127) // 128

    nc = _get_nc(tch)
    in_maps = _make_in_maps(tch, x, W, b, start, end, groups, los)
    res = bass_utils.run_bass_kernel_spmd(
        nc,
        in_maps,
        core_ids=list(range(N_CORES)),
        trace=trace,
        trace_cores=trace_cores,
    )
    out = np.empty((B, S, E), np.float32)
    for g in range(N_CORES):
        out[:, groups[g], :] = (
            res.results[g]["out"].astype(np.float32).reshape(B, SQ, E)
        )
    return out, res


def kernel(x, W, b, start, end):
    out, _ = run(x, W, b, start, end, trace=False)
    return out
